# revision 1
# baseline (speedup 1.0000x reference)
"""2-layer GAT + FC tail on 8 Trainium2 NeuronCores (Bass/Tile).

Strategy: sort edges by destination, shard destination blocks (128 nodes)
across the 8 cores so each core fully aggregates its own nodes (no
all-reduce of node features needed).  Per layer, a fused per-node table
[h | a_src | a_dst] is computed from the node shard and all-gathered; the
edge phase gathers source rows with batched indirect DMAs, computes
attention numerators p = exp(leaky_relu(a_src[src]+a_dst[dst])), and
accumulates [sum_e p*h[src] | sum_e p] per destination block with one-hot
matmuls into PSUM.  The flush normalizes by the denominator (softmax),
applies bias+ELU and projects the next layer's table.  fc1 is row-sharded
to match each core's node range and accumulated into one [84,1] PSUM,
followed by a tiny all-reduce and the replicated fc2/fc3/log-softmax tail.
"""

import math
import numpy as np

P = 128
NC_CORES = 8
NEG_SLOPE = 0.2
SENT_VAL = -60000.0  # sentinel a_src/a_dst => p = exp(lrelu(-huge)) == 0

_cache = {}


def kernel(**inputs):
    out, _res = _run(inputs, trace=False)
    return out


def _prep_host(inputs, K1, K2):
    """All host-side index/layout prep. Returns (cfg, shared, per_core)."""
    x = np.asarray(inputs["x"], np.float32)
    ei = np.asarray(inputs["edge_index"])
    W1 = np.asarray(inputs["W1"], np.float32)
    as1 = np.asarray(inputs["att_src1"], np.float32)
    ad1 = np.asarray(inputs["att_dst1"], np.float32)
    b1 = np.asarray(inputs["b1"], np.float32)
    W2 = np.asarray(inputs["W2"], np.float32)
    as2 = np.asarray(inputs["att_src2"], np.float32)
    ad2 = np.asarray(inputs["att_dst2"], np.float32)
    b2 = np.asarray(inputs["b2"], np.float32)
    fc1_w = np.asarray(inputs["fc1_w"], np.float32)
    fc1_b = np.asarray(inputs["fc1_b"], np.float32)
    fc2_w = np.asarray(inputs["fc2_w"], np.float32)
    fc2_b = np.asarray(inputs["fc2_b"], np.float32)
    fc3_w = np.asarray(inputs["fc3_w"], np.float32)
    fc3_b = np.asarray(inputs["fc3_b"], np.float32)

    N, F = x.shape
    H1, D1 = as1.shape
    D2 = W2.shape[1]
    FH = H1 * D1  # 128
    assert F == P and FH == P
    NF1 = fc1_w.shape[1]  # 84
    NF2 = fc2_w.shape[1]  # 24
    NF3 = fc3_w.shape[1]  # 2

    NPC = int(math.ceil(N / (NC_CORES * P))) * P  # nodes per core (padded)
    NBLK = NPC // P                               # dst blocks per core
    NPAD = NC_CORES * NPC
    SENT = NPAD                                   # sentinel table row

    # ---- edges: add self loops, sort by dst ----
    src = np.concatenate([ei[0], np.arange(N)]).astype(np.int64)
    dst = np.concatenate([ei[1], np.arange(N)]).astype(np.int64)
    order = np.argsort(dst, kind="stable")
    src_s = src[order].astype(np.int32)
    dst_s = dst[order].astype(np.int32)

    nblk_tot = NPAD // P
    blk_id = dst_s // P
    blk_cnt = np.bincount(blk_id, minlength=nblk_tot)
    T_BLK = int(math.ceil(blk_cnt.max() / P))     # subtiles per block (uniform)
    NSUB = NBLK * T_BLK                           # subtiles per core
    blk_start = np.concatenate([[0], np.cumsum(blk_cnt)]).astype(np.int64)

    idx_src = np.full((NC_CORES, NSUB * P), SENT, np.int32)
    idx_dst = np.full((NC_CORES, NSUB * P), SENT, np.int32)
    dstl = np.zeros((NC_CORES, NSUB * P), np.float32)
    for c in range(NC_CORES):
        for bl in range(NBLK):
            g = c * NBLK + bl
            lo, hi = blk_start[g], blk_start[g + 1]
            cnt = hi - lo
            base = bl * T_BLK * P
            idx_src[c, base:base + cnt] = src_s[lo:hi]
            idx_dst[c, base:base + cnt] = dst_s[lo:hi]
            dstl[c, base:base + cnt] = (dst_s[lo:hi] - g * P).astype(np.float32)
    # [p, s] layout: element (partition p, subtile s) = slot s*128+p
    idx_src = np.ascontiguousarray(idx_src.reshape(NC_CORES, NSUB, P).transpose(0, 2, 1))
    idx_dst = np.ascontiguousarray(idx_dst.reshape(NC_CORES, NSUB, P).transpose(0, 2, 1))
    dstl = np.ascontiguousarray(dstl.reshape(NC_CORES, NSUB, P).transpose(0, 2, 1))

    # ---- weights ----
    asrc_col = np.stack([W1[:, h * D1:(h + 1) * D1] @ as1[h] for h in range(H1)], axis=1)
    adst_col = np.stack([W1[:, h * D1:(h + 1) * D1] @ ad1[h] for h in range(H1)], axis=1)
    W1cat = np.concatenate([W1, asrc_col, adst_col], axis=1).astype(np.float32)  # [128, 144]
    W2cat = np.concatenate([W2, W2 @ as2[0][:, None], W2 @ ad2[0][:, None]],
                           axis=1).astype(np.float32)                            # [128, D2+2]

    # ---- fc1 row shard, permuted to (block, feat, partition) chunk order ----
    fpad = np.zeros((NPAD, D2, NF1), np.float32)
    fpad[:N] = fc1_w.reshape(N, D2, NF1)
    # [core, block*P, D2*NF1]: row (b,p) holds fc1_w rows for node(b,p), kk-major
    Wfc = (fpad.reshape(NC_CORES, NBLK, P, D2 * NF1)
           .reshape(NC_CORES, NBLK * P, D2 * NF1).astype(np.float16))
    NCHUNK = NBLK * D2

    # ---- per-core node shard of x ----
    xpad = np.zeros((NPAD, P), np.float32)
    xpad[:N] = x

    cfg = dict(N=N, F=F, H1=H1, D1=D1, D2=D2, FH=FH, NF1=NF1, NF2=NF2, NF3=NF3,
               NPC=NPC, NBLK=NBLK, NPAD=NPAD, SENT=SENT, T_BLK=T_BLK, NSUB=NSUB,
               NCHUNK=NCHUNK, K1=K1, K2=K2)

    shared = dict(
        W1cat=W1cat, W2cat=W2cat,
        b1t=np.ascontiguousarray(np.broadcast_to(b1, (P, FH)).astype(np.float32)),
        b2t=np.ascontiguousarray(np.broadcast_to(b2, (P, D2)).astype(np.float32)),
        fc1_b=fc1_b.reshape(NF1, 1).astype(np.float32),
        fc2_w=fc2_w.astype(np.float32),
        fc2_b=fc2_b.reshape(NF2, 1).astype(np.float32),
        fc3_w=fc3_w.astype(np.float32),
        fc3_b=fc3_b.reshape(NF3, 1).astype(np.float32),
    )
    per_core = []
    for c in range(NC_CORES):
        per_core.append(dict(
            x_shard=np.ascontiguousarray(xpad[c * NPC:(c + 1) * NPC]),
            idx_src=idx_src[c], idx_dst=idx_dst[c], dstf=dstl[c],
            wfc=np.ascontiguousarray(Wfc[c]),
        ))
    return cfg, shared, per_core


def _build(cfg):
    """Build the Bass/Tile program. Returns nc."""
    import concourse.bacc as bacc
    import concourse.mybir as mybir
    import concourse.tile as tile
    import concourse.bass as bass
    from concourse.masks import make_identity

    f32 = mybir.dt.float32
    f16 = mybir.dt.float16
    i32 = mybir.dt.int32
    AF = mybir.ActivationFunctionType
    OP = mybir.AluOpType

    FH, D2, H1, D1 = cfg["FH"], cfg["D2"], cfg["H1"], cfg["D1"]
    NF1, NF2, NF3 = cfg["NF1"], cfg["NF2"], cfg["NF3"]
    NPC, NBLK, NPAD = cfg["NPC"], cfg["NBLK"], cfg["NPAD"]
    T_BLK, NSUB, NCHUNK = cfg["T_BLK"], cfg["NSUB"], cfg["NCHUNK"]
    K1, K2 = cfg["K1"], cfg["K2"]
    C1 = FH + H1          # 136: gathered row = [h | a_src]
    R1 = FH + 2 * H1      # 144: full table row
    C2 = D2 + 1           # 9
    R2 = D2 + 4           # 12 (padded row)
    RG = [list(range(NC_CORES))]

    nc = bacc.Bacc("TRN2", target_bir_lowering=False, debug=False,
                   num_devices=NC_CORES)

    # ---- I/O ----
    x_sh = nc.dram_tensor("x_shard", [NPC, P], f32, kind="ExternalInput").ap()
    ap_isrc = nc.dram_tensor("idx_src", [P, NSUB], i32, kind="ExternalInput").ap()
    ap_idst = nc.dram_tensor("idx_dst", [P, NSUB], i32, kind="ExternalInput").ap()
    ap_dstf = nc.dram_tensor("dstf", [P, NSUB], f32, kind="ExternalInput").ap()
    ap_w1 = nc.dram_tensor("W1cat", [P, R1], f32, kind="ExternalInput").ap()
    ap_w2 = nc.dram_tensor("W2cat", [P, D2 + 2], f32, kind="ExternalInput").ap()
    ap_b1 = nc.dram_tensor("b1t", [P, FH], f32, kind="ExternalInput").ap()
    ap_b2 = nc.dram_tensor("b2t", [P, D2], f32, kind="ExternalInput").ap()
    ap_wfc = nc.dram_tensor("wfc", [NBLK * P, D2 * NF1], f16, kind="ExternalInput").ap()
    ap_f1b = nc.dram_tensor("fc1_b", [NF1, 1], f32, kind="ExternalInput").ap()
    ap_f2w = nc.dram_tensor("fc2_w", [NF1, NF2], f32, kind="ExternalInput").ap()
    ap_f2b = nc.dram_tensor("fc2_b", [NF2, 1], f32, kind="ExternalInput").ap()
    ap_f3w = nc.dram_tensor("fc3_w", [NF2, NF3], f32, kind="ExternalInput").ap()
    ap_f3b = nc.dram_tensor("fc3_b", [NF3, 1], f32, kind="ExternalInput").ap()
    ap_y = nc.dram_tensor("y", [1, NF3], f32, kind="ExternalOutput").ap()
    ap_dz1 = nc.dram_tensor("dbg_z1", [NF1, 1], f32, kind="ExternalOutput").ap()
    ap_dzr = nc.dram_tensor("dbg_zr", [NF1, 1], f32, kind="ExternalOutput").ap()

    with tile.TileContext(nc) as tc:
        with tc.tile_pool(name="const", bufs=1) as cp, \
             tc.tile_pool(name="dram", bufs=1, space="DRAM") as dp:

            ident = cp.tile([P, P], f32)
            make_identity(nc, ident[:])
            iota_i = cp.tile([P, P], i32)
            nc.gpsimd.iota(iota_i[:], pattern=[[1, P]], base=0, channel_multiplier=0)
            iota_f = cp.tile([P, P], f32)
            nc.vector.tensor_copy(iota_f[:], iota_i[:])

            w1c = cp.tile([P, R1], f32)
            nc.sync.dma_start(out=w1c[:], in_=ap_w1)
            w2c = cp.tile([P, D2 + 2], f32)
            nc.sync.dma_start(out=w2c[:], in_=ap_w2)
            b1t = cp.tile([P, FH], f32)
            nc.sync.dma_start(out=b1t[:], in_=ap_b1)
            b2t = cp.tile([P, D2], f32)
            nc.sync.dma_start(out=b2t[:], in_=ap_b2)

            # DRAM tables
            tab1_sh = dp.tile([NPC, R1], f16)
            tab1 = dp.tile([NPAD + 1, R1], f16)
            tab2_sh = dp.tile([NPC, R2], f16)
            tab2 = dp.tile([NPAD + 1, R2], f16)
            ar_in = dp.tile([NF1, 1], f32)
            ar_out = dp.tile([NF1, 1], f32)

            # ---------- phase 1: per-node table for layer 1 ----------
            with tc.tile_pool(name="p1", bufs=3) as p1, \
                 tc.tile_pool(name="p1ps", bufs=2, space="PSUM") as p1ps:
                for i in range(NBLK):
                    xt = p1.tile([P, P], f32, tag="xt")
                    nc.sync.dma_start(out=xt[:], in_=x_sh[i * P:(i + 1) * P, :])
                    xps = p1ps.tile([P, P], f32, tag="xps")
                    nc.tensor.transpose(xps[:], xt[:], ident[:])
                    xT = p1.tile([P, P], f32, tag="xT")
                    nc.vector.tensor_copy(xT[:], xps[:])
                    hps = p1ps.tile([P, R1], f32, tag="hps")
                    nc.tensor.matmul(hps[:], lhsT=xT[:], rhs=w1c[:],
                                     start=True, stop=True)
                    t1 = p1.tile([P, R1], f16, tag="t1")
                    nc.vector.tensor_copy(t1[:], hps[:])
                    nc.sync.dma_start(out=tab1_sh[i * P:(i + 1) * P, :], in_=t1[:])

            tc.strict_bb_all_engine_barrier()
            nc.gpsimd.collective_compute(
                "AllGather", mybir.AluOpType.bypass, replica_groups=RG,
                ins=[tab1_sh[:].opt()], outs=[tab1[0:NPAD, :].opt()])
            # sentinel row: h part = 0, a_src/a_dst parts = SENT_VAL
            srow1 = cp.tile([1, R1], f16)
            nc.vector.memset(srow1[:, 0:FH], 0.0)
            nc.vector.memset(srow1[:, FH:R1], SENT_VAL)
            nc.sync.dma_start(out=tab1[NPAD:NPAD + 1, :], in_=srow1[:])
            tc.strict_bb_all_engine_barrier()

            # ---------- fc1 psum (accumulated during layer-2 flushes) ----------
            with tc.tile_pool(name="fcps", bufs=1, space="PSUM") as fcps:
                ps_fc = fcps.tile([NF1, 1], f32)

                # ---------- layer-1 edge phase ----------
                nsup1 = (NSUB + K1 - 1) // K1
                with tc.tile_pool(name="l1i", bufs=2) as l1i, \
                     tc.tile_pool(name="l1g", bufs=2) as l1g, \
                     tc.tile_pool(name="l1w", bufs=2) as l1w, \
                     tc.tile_pool(name="l1f", bufs=2) as l1f, \
                     tc.tile_pool(name="l1ps", bufs=2, space="PSUM") as l1ps, \
                     tc.tile_pool(name="l1fps", bufs=2, space="PSUM") as l1fps:
                    cur = None
                    for t in range(nsup1):
                        k = min(K1, NSUB - t * K1)
                        s0 = t * K1
                        its = l1i.tile([P, k], i32, tag="isrc")
                        nc.sync.dma_start(out=its[:], in_=ap_isrc[:, s0:s0 + k])
                        itd = l1i.tile([P, k], i32, tag="idst")
                        nc.sync.dma_start(out=itd[:], in_=ap_idst[:, s0:s0 + k])
                        itf = l1i.tile([P, k], f32, tag="dstf")
                        nc.sync.dma_start(out=itf[:], in_=ap_dstf[:, s0:s0 + k])

                        g = l1g.tile([P, k * C1], f16, tag="g")
                        nc.gpsimd.indirect_dma_start(
                            out=g[:], out_offset=None, in_=tab1[:],
                            in_offset=bass.IndirectOffsetOnAxis(ap=its[:], axis=0))
                        ad = l1g.tile([P, k * H1], f16, tag="ad")
                        nc.gpsimd.indirect_dma_start(
                            out=ad[:], out_offset=None, in_=tab1[:],
                            in_offset=bass.IndirectOffsetOnAxis(ap=itd[:], axis=0),
                            element_offset=C1)

                        gv = g[:].rearrange("p (k c) -> p k c", k=k)
                        adv = ad[:].rearrange("p (k c) -> p k c", k=k)

                        mt = l1w.tile([P, k * P], f16, tag="mt")
                        for j in range(k):
                            nc.vector.tensor_scalar(
                                out=mt[:, j * P:(j + 1) * P], in0=iota_f[:],
                                scalar1=itf[:, j:j + 1], scalar2=None,
                                op0=OP.is_equal)

                        te = l1w.tile([P, k * H1], f32, tag="te")
                        tev = te[:].rearrange("p (k c) -> p k c", k=k)
                        nc.vector.tensor_tensor(out=tev, in0=gv[:, :, FH:C1],
                                                in1=adv, op=OP.add)
                        tl = l1w.tile([P, k * H1], f32, tag="tl")
                        nc.vector.tensor_scalar_mul(tl[:], te[:], NEG_SLOPE)
                        tm = l1w.tile([P, k * H1], f32, tag="tm")
                        nc.vector.tensor_tensor(out=tm[:], in0=tl[:], in1=te[:],
                                                op=OP.max)
                        mp = l1w.tile([P, k * C1], f16, tag="mp")
                        mpv = mp[:].rearrange("p (k c) -> p k c", k=k)
                        nc.scalar.activation(out=mpv[:, :, FH:C1],
                                             in_=tm[:].rearrange("p (k c) -> p k c", k=k),
                                             func=AF.Exp)
                        nc.vector.tensor_tensor(
                            out=mpv[:, :, 0:FH], in0=gv[:, :, 0:FH],
                            in1=mpv[:, :, FH:C1][:, :, :, None]
                                .to_broadcast([P, k, H1, D1]),
                            op=OP.mult)

                        for j in range(k):
                            s = s0 + j
                            b = s // T_BLK
                            pos = s % T_BLK
                            if pos == 0:
                                cur = l1ps.tile([P, C1], f32, tag="pb")
                            nc.tensor.matmul(
                                cur[:], lhsT=mt[:, j * P:(j + 1) * P],
                                rhs=mp[:, j * C1:(j + 1) * C1],
                                start=(pos == 0), stop=(pos == T_BLK - 1),
                                skip_group_check=True)
                            if pos == T_BLK - 1:
                                # ---- flush block b -> tab2 rows ----
                                den = l1f.tile([P, H1], f32, tag="den")
                                nc.vector.tensor_scalar_max(den[:], cur[:, FH:C1], 1e-30)
                                rec = l1f.tile([P, H1], f32, tag="rec")
                                nc.vector.reciprocal(rec[:], den[:])
                                u = l1f.tile([P, FH], f32, tag="u")
                                nc.vector.tensor_tensor(
                                    out=u[:].rearrange("p (h d) -> p h d", h=H1),
                                    in0=cur[:, 0:FH].rearrange("p (h d) -> p h d", h=H1),
                                    in1=rec[:][:, :, None].to_broadcast([P, H1, D1]),
                                    op=OP.mult)
                                u2 = l1f.tile([P, FH], f32, tag="u2")
                                nc.vector.tensor_tensor(out=u2[:], in0=u[:],
                                                        in1=b1t[:], op=OP.add)
                                em = l1f.tile([P, FH], f32, tag="em")
                                nc.vector.tensor_scalar_min(em[:], u2[:], 0.0)
                                ee = l1f.tile([P, FH], f32, tag="ee")
                                nc.scalar.activation(out=ee[:], in_=em[:], func=AF.Exp)
                                er = l1f.tile([P, FH], f32, tag="er")
                                nc.vector.tensor_scalar_max(er[:], u2[:], 0.0)
                                h2a = l1f.tile([P, FH], f32, tag="h2a")
                                nc.vector.tensor_tensor(out=h2a[:], in0=ee[:],
                                                        in1=er[:], op=OP.add)
                                h2b = l1f.tile([P, FH], f32, tag="h2b")
                                nc.vector.tensor_scalar_add(h2b[:], h2a[:], -1.0)
                                tp = l1fps.tile([P, P], f32, tag="tp")
                                nc.tensor.transpose(tp[:], h2b[:], ident[:])
                                h2T = l1f.tile([P, P], f32, tag="h2T")
                                nc.vector.tensor_copy(h2T[:], tp[:])
                                pj = l1fps.tile([P, D2 + 2], f32, tag="pj")
                                nc.tensor.matmul(pj[:], lhsT=h2T[:], rhs=w2c[:],
                                                 start=True, stop=True,
                                                 skip_group_check=True)
                                t2 = l1f.tile([P, R2], f16, tag="t2")
                                nc.vector.tensor_copy(t2[:, 0:D2 + 2], pj[:])
                                nc.vector.memset(t2[:, D2 + 2:R2], 0.0)
                                nc.sync.dma_start(
                                    out=tab2_sh[b * P:(b + 1) * P, :],
                                    in_=t2[:])

                tc.strict_bb_all_engine_barrier()
                nc.gpsimd.collective_compute(
                    "AllGather", mybir.AluOpType.bypass, replica_groups=RG,
                    ins=[tab2_sh[:].opt()], outs=[tab2[0:NPAD, :].opt()])
                srow2 = cp.tile([1, R2], f16)
                nc.vector.memset(srow2[:, 0:D2], 0.0)
                nc.vector.memset(srow2[:, D2:R2], SENT_VAL)
                nc.sync.dma_start(out=tab2[NPAD:NPAD + 1, :], in_=srow2[:])
                tc.strict_bb_all_engine_barrier()

                # ---------- layer-2 edge phase + fc1 ----------
                nsup2 = (NSUB + K2 - 1) // K2
                with tc.tile_pool(name="l2i", bufs=2) as l2i, \
                     tc.tile_pool(name="l2g", bufs=2) as l2g, \
                     tc.tile_pool(name="l2w", bufs=2) as l2w, \
                     tc.tile_pool(name="l2f", bufs=2) as l2f, \
                     tc.tile_pool(name="l2wt", bufs=3) as l2wt, \
                     tc.tile_pool(name="l2ps", bufs=2, space="PSUM") as l2ps:
                    cur2 = None
                    for t in range(nsup2):
                        k = min(K2, NSUB - t * K2)
                        s0 = t * K2
                        its = l2i.tile([P, k], i32, tag="isrc")
                        nc.sync.dma_start(out=its[:], in_=ap_isrc[:, s0:s0 + k])
                        itd = l2i.tile([P, k], i32, tag="idst")
                        nc.sync.dma_start(out=itd[:], in_=ap_idst[:, s0:s0 + k])
                        itf = l2i.tile([P, k], f32, tag="dstf")
                        nc.sync.dma_start(out=itf[:], in_=ap_dstf[:, s0:s0 + k])

                        g2 = l2g.tile([P, k * C2], f16, tag="g2")
                        nc.gpsimd.indirect_dma_start(
                            out=g2[:], out_offset=None, in_=tab2[:],
                            in_offset=bass.IndirectOffsetOnAxis(ap=its[:], axis=0))
                        ad2 = l2g.tile([P, k], f16, tag="ad2")
                        nc.gpsimd.indirect_dma_start(
                            out=ad2[:], out_offset=None, in_=tab2[:],
                            in_offset=bass.IndirectOffsetOnAxis(ap=itd[:], axis=0),
                            element_offset=C2)

                        g2v = g2[:].rearrange("p (k c) -> p k c", k=k)

                        mt2 = l2w.tile([P, k * P], f16, tag="mt2")
                        for j in range(k):
                            nc.vector.tensor_scalar(
                                out=mt2[:, j * P:(j + 1) * P], in0=iota_f[:],
                                scalar1=itf[:, j:j + 1], scalar2=None,
                                op0=OP.is_equal)

                        te2 = l2w.tile([P, k], f32, tag="te2")
                        nc.vector.tensor_tensor(
                            out=te2[:].rearrange("p (k c) -> p k c", k=k),
                            in0=g2v[:, :, D2:C2],
                            in1=ad2[:].rearrange("p (k c) -> p k c", k=k),
                            op=OP.add)
                        tl2 = l2w.tile([P, k], f32, tag="tl2")
                        nc.vector.tensor_scalar_mul(tl2[:], te2[:], NEG_SLOPE)
                        tm2 = l2w.tile([P, k], f32, tag="tm2")
                        nc.vector.tensor_tensor(out=tm2[:], in0=tl2[:], in1=te2[:],
                                                op=OP.max)
                        mp2 = l2w.tile([P, k * C2], f16, tag="mp2")
                        mpv2 = mp2[:].rearrange("p (k c) -> p k c", k=k)
                        nc.scalar.activation(
                            out=mpv2[:, :, D2:C2],
                            in_=tm2[:].rearrange("p (k c) -> p k c", k=k),
                            func=AF.Exp)
                        nc.vector.tensor_tensor(
                            out=mpv2[:, :, 0:D2], in0=g2v[:, :, 0:D2],
                            in1=mpv2[:, :, D2:C2][:, :, :, None]
                                .to_broadcast([P, k, 1, D2]),
                            op=OP.mult)

                        for j in range(k):
                            s = s0 + j
                            b = s // T_BLK
                            pos = s % T_BLK
                            if pos == 0:
                                cur2 = l2ps.tile([P, C2], f32, tag="pb2")
                            nc.tensor.matmul(
                                cur2[:], lhsT=mt2[:, j * P:(j + 1) * P],
                                rhs=mp2[:, j * C2:(j + 1) * C2],
                                start=(pos == 0), stop=(pos == T_BLK - 1),
                                skip_group_check=True)
                            if pos == T_BLK - 1:
                                # ---- flush block b: h3 block + fc1 chunks ----
                                den2 = l2f.tile([P, 1], f32, tag="den2")
                                nc.vector.tensor_scalar_max(den2[:], cur2[:, D2:C2], 1e-30)
                                rec2 = l2f.tile([P, 1], f32, tag="rec2")
                                nc.vector.reciprocal(rec2[:], den2[:])
                                u_2 = l2f.tile([P, D2], f32, tag="u_2")
                                nc.vector.tensor_tensor(
                                    out=u_2[:], in0=cur2[:, 0:D2],
                                    in1=rec2[:].to_broadcast([P, D2]),
                                    op=OP.mult)
                                ub2 = l2f.tile([P, D2], f32, tag="ub2")
                                nc.vector.tensor_tensor(out=ub2[:], in0=u_2[:],
                                                        in1=b2t[:], op=OP.add)
                                em2 = l2f.tile([P, D2], f32, tag="em2")
                                nc.vector.tensor_scalar_min(em2[:], ub2[:], 0.0)
                                ee2 = l2f.tile([P, D2], f32, tag="ee2")
                                nc.scalar.activation(out=ee2[:], in_=em2[:], func=AF.Exp)
                                er2 = l2f.tile([P, D2], f32, tag="er2")
                                nc.vector.tensor_scalar_max(er2[:], ub2[:], 0.0)
                                h3a = l2f.tile([P, D2], f32, tag="h3a")
                                nc.vector.tensor_tensor(out=h3a[:], in0=ee2[:],
                                                        in1=er2[:], op=OP.add)
                                h3f = l2f.tile([P, D2], f16, tag="h3f")
                                nc.vector.tensor_scalar_add(h3f[:], h3a[:], -1.0)
                                wt = l2wt.tile([P, D2 * NF1], f16, tag="wfc")
                                nc.sync.dma_start(
                                    out=wt[:],
                                    in_=ap_wfc[b * P:(b + 1) * P, :])
                                for kk in range(D2):
                                    ch = b * D2 + kk
                                    wcp = l2wt.tile([P, NF1], f16, tag="wcp")
                                    nc.vector.tensor_copy(
                                        wcp[:], wt[:, kk * NF1:(kk + 1) * NF1])
                                    nc.tensor.matmul(
                                        ps_fc[:], lhsT=wcp[:],
                                        rhs=h3f[:, kk:kk + 1],
                                        start=(ch == 0), stop=(ch == NCHUNK - 1),
                                        skip_group_check=True)

                # ---------- fc tail ----------
                with tc.tile_pool(name="tail", bufs=1) as tp_, \
                     tc.tile_pool(name="tailps", bufs=1, space="PSUM") as tps_:
                    z1 = tp_.tile([NF1, 1], f32)
                    nc.vector.tensor_copy(z1[:], ps_fc[:])
                    nc.sync.dma_start(out=ar_in[:], in_=z1[:])
                    nc.sync.dma_start(out=ap_dz1, in_=z1[:])
                    tc.strict_bb_all_engine_barrier()
                    nc.gpsimd.collective_compute(
                        "AllReduce", mybir.AluOpType.add, replica_groups=RG,
                        ins=[ar_in[:].opt()], outs=[ar_out[:].opt()])
                    tc.strict_bb_all_engine_barrier()
                    zr = tp_.tile([NF1, 1], f32)
                    nc.sync.dma_start(out=zr[:], in_=ar_out[:])
                    nc.sync.dma_start(out=ap_dzr, in_=zr[:])
                    f1b = tp_.tile([NF1, 1], f32)
                    nc.sync.dma_start(out=f1b[:], in_=ap_f1b)
                    f2w = tp_.tile([NF1, NF2], f32)
                    nc.sync.dma_start(out=f2w[:], in_=ap_f2w)
                    f2b = tp_.tile([NF2, 1], f32)
                    nc.sync.dma_start(out=f2b[:], in_=ap_f2b)
                    f3w = tp_.tile([NF2, NF3], f32)
                    nc.sync.dma_start(out=f3w[:], in_=ap_f3w)
                    f3b = tp_.tile([NF3, 1], f32)
                    nc.sync.dma_start(out=f3b[:], in_=ap_f3b)

                    def elu_col(src_t, n):
                        zb = tp_.tile([n, 1], f32, name=f"zb{n}")
                        nc.vector.tensor_scalar_min(zb[:], src_t[:], 0.0)
                        ze = tp_.tile([n, 1], f32, name=f"ze{n}")
                        nc.scalar.activation(out=ze[:], in_=zb[:], func=AF.Exp)
                        zm = tp_.tile([n, 1], f32, name=f"zm{n}")
                        nc.vector.tensor_scalar_max(zm[:], src_t[:], 0.0)
                        zs = tp_.tile([n, 1], f32, name=f"zs{n}")
                        nc.vector.tensor_tensor(out=zs[:], in0=ze[:], in1=zm[:],
                                                op=OP.add)
                        zo = tp_.tile([n, 1], f32, name=f"zo{n}")
                        nc.vector.tensor_scalar_add(zo[:], zs[:], -1.0)
                        return zo

                    za = tp_.tile([NF1, 1], f32)
                    nc.vector.tensor_tensor(out=za[:], in0=zr[:], in1=f1b[:], op=OP.add)
                    z1e = elu_col(za, NF1)
                    p2 = tps_.tile([NF2, 1], f32, tag="p2")
                    nc.tensor.matmul(p2[:], lhsT=f2w[:], rhs=z1e[:],
                                     start=True, stop=True, skip_group_check=True)
                    z2 = tp_.tile([NF2, 1], f32)
                    nc.vector.tensor_tensor(out=z2[:], in0=p2[:], in1=f2b[:], op=OP.add)
                    z2e = elu_col(z2, NF2)
                    p3 = tps_.tile([NF3, 1], f32, tag="p3")
                    nc.tensor.matmul(p3[:], lhsT=f3w[:], rhs=z2e[:],
                                     start=True, stop=True, skip_group_check=True)
                    z3 = tp_.tile([NF3, 1], f32)
                    nc.vector.tensor_tensor(out=z3[:], in0=p3[:], in1=f3b[:], op=OP.add)
                    # transpose [NF3,1] -> [1,NF3], then log-softmax
                    ptr = tps_.tile([1, NF3], f32, tag="ptr")
                    nc.tensor.matmul(ptr[:], lhsT=z3[:], rhs=ident[:NF3, :NF3],
                                     is_transpose=True, skip_group_check=True)
                    z4 = tp_.tile([1, NF3], f32)
                    nc.vector.tensor_copy(z4[:], ptr[:])
                    ex = tp_.tile([1, NF3], f32)
                    nc.scalar.activation(out=ex[:], in_=z4[:], func=AF.Exp)
                    ssum = tp_.tile([1, 1], f32)
                    nc.vector.tensor_reduce(ssum[:], ex[:],
                                            axis=mybir.AxisListType.X, op=OP.add)
                    lnt = tp_.tile([1, 1], f32)
                    nc.scalar.activation(out=lnt[:], in_=ssum[:], func=AF.Ln)
                    yt = tp_.tile([1, NF3], f32)
                    nc.vector.tensor_tensor(out=yt[:], in0=z4[:],
                                            in1=lnt[:].to_broadcast([1, NF3]),
                                            op=OP.subtract)
                    nc.sync.dma_start(out=ap_y, in_=yt[:])

    nc.compile()
    return nc


def _run(inputs, trace=False, K1=64, K2=128):
    from concourse import bass_utils

    cfg, shared, per_core = _prep_host(inputs, K1, K2)

    key = (cfg["N"], cfg["NSUB"], cfg["T_BLK"], K1, K2)
    if key not in _cache:
        _cache[key] = _build(cfg)
    nc = _cache[key]

    in_maps = []
    for c in range(NC_CORES):
        pc = per_core[c]
        in_maps.append({
            "x_shard": pc["x_shard"],
            "idx_src": pc["idx_src"], "idx_dst": pc["idx_dst"], "dstf": pc["dstf"],
            "W1cat": shared["W1cat"], "W2cat": shared["W2cat"],
            "b1t": shared["b1t"], "b2t": shared["b2t"],
            "wfc": pc["wfc"],
            "fc1_b": shared["fc1_b"], "fc2_w": shared["fc2_w"],
            "fc2_b": shared["fc2_b"], "fc3_w": shared["fc3_w"],
            "fc3_b": shared["fc3_b"],
        })

    res = bass_utils.run_bass_kernel_spmd(
        nc, in_maps, core_ids=list(range(NC_CORES)), trace=trace)
    y = res.results[0]["y"].astype(np.float32)
    return y, res

